# revision 18
# baseline (speedup 1.0000x reference)
"""Trainium2 Bass kernel for DiffusioUnpool (gnn_message_passing).

Computes, for a graph with N=12288 nodes, F=128 features, E=393216 COO edges:
    x_zero    = scatter(fea via perm)                     [N, F]
    atte_zero = scatter(tiled node_atte_coffe via perm)   [N]
    A         = coo_sum(edges) + I                        (dense adjacency)
    dinv      = 1/sqrt(A.sum(axis=1))
    x         = dinv * (A @ (x_zero * dinv))
    atte      = dinv * (A @ (atte_zero * dinv))

Strategy (row-sharded across 8 cores, 1536 rows each):
    out[r] = dinv_r * sum_dst A[r,dst] * z[dst]  +  dinv_r^2 * xa[r]
with z = [x_zero | atte_zero] * dinv built on-device in fp16 and kept
SBUF-resident (3.2MB), and the dense A^T streamed from DRAM in fp16 as the
matmul stationary operand: per output row-tile, one contiguous 3.1MB DMA
brings all 96 [128x128] A^T K-tiles, and the PE accumulates 96 fp16 matmuls
(N=129) into a fp32 PSUM bank.  Row sums (degree+1) are reduced on-device
from a padded ELL layout of attr; the epilogue rescales by the row dinv and
adds the identity term in fp32.  This keeps the kernel entirely on the
HWDGE DMA path + PE: no software-DGE gathers (the Q7 descriptor-generation
rate, ~8ns/descriptor, made per-edge gathering the bottleneck).
"""

import os
import sys

import numpy as np

for _p in ("/opt/trn_rl_repo", "/root/.axon_site/_ro/trn_rl_repo"):
    if os.path.isdir(_p) and _p not in sys.path:
        sys.path.append(_p)

import concourse.bacc as bacc
import concourse.bass as bass
import concourse.mybir as mybir
import concourse.tile as tile
from concourse.bass_utils import run_bass_kernel_spmd

FP32 = mybir.dt.float32
FP16 = mybir.dt.float16

N = 12288          # all_node_num
F = 128            # feature dim
FA = F + 1         # features + atte channel
NCORES = 8
P = 128            # partitions
RPC = N // NCORES  # rows per core = 1536
RT = RPC // P      # row tiles per core = 12
GT = N // P        # node tiles = 96

# Stash of the last BassKernelResults (test.py reads .exec_time_ns)
LAST_RESULTS = None
# Extra kwargs test.py can inject into run_bass_kernel_spmd (e.g. trace)
RUN_KWARGS = {}


# ---------------------------------------------------------------------------
# Host-side preparation: scatter, dense A^T tiles, ELL degree layout
# ---------------------------------------------------------------------------

def host_prep(fea, perm, encoder_edge_index, encoder_edge_attr, node_atte_coffe,
              all_node_num, batch_size):
    n = int(all_node_num)
    b = int(batch_size)
    assert n == N
    fea = np.asarray(fea, dtype=np.float32)
    perm = np.asarray(perm).astype(np.int64)
    eidx = np.asarray(encoder_edge_index).astype(np.int64)
    attr = np.asarray(encoder_edge_attr, dtype=np.float32)
    natte = np.asarray(node_atte_coffe, dtype=np.float32)

    n_perm, f = fea.shape
    assert f == F
    node_num = natte.shape[0] // b
    swn = n_perm // natte.shape[0]

    # unpool scatters
    x_zero = np.zeros((N, F), dtype=np.float32)
    x_zero[perm] = fea
    win = np.broadcast_to(natte.reshape(b, 1, node_num),
                          (b, swn, node_num)).reshape(-1).astype(np.float32)
    atte_zero = np.zeros((N,), dtype=np.float32)
    atte_zero[perm] = win

    src = eidx[0]
    dst = eidx[1]
    E = src.shape[0]

    # --- ELL attr layout for on-device row sums (t-major: g = t*P + p) ---
    deg = np.bincount(src, minlength=N)
    deg_pad = max(4, int(-(-int(deg.max()) // 8) * 8))
    o1 = np.argsort(src, kind="stable")
    ssrc = src[o1]
    row_starts = np.zeros(N, dtype=np.int64)
    row_starts[1:] = np.cumsum(deg)[:-1]
    pos1 = np.arange(E) - row_starts[ssrc]
    ell = np.zeros((N, deg_pad), dtype=np.float32)
    ell[ssrc, pos1] = attr[o1]
    ell_dev = np.ascontiguousarray(
        ell.reshape(GT, P, deg_pad).transpose(1, 0, 2)).astype(np.float16)

    # [x_zero | atte_zero] in t-major node-tile layout for the z build
    xat = np.zeros((P, GT, FA), dtype=np.float32)
    xat[:, :, :F] = x_zero.reshape(GT, P, F).transpose(1, 0, 2)
    xat[:, :, F] = atte_zero.reshape(GT, P).T

    # --- dense A^T, fp16, tiled per core: at[rt, p_dst, kt, f_src] ---
    A32 = np.zeros((N, N), dtype=np.float32)
    np.add.at(A32, (src, dst), attr)
    A16 = A32.astype(np.float16)
    del A32
    # [rt_g, f_src, kt, p_dst] view of A[row, dst]
    A4 = A16.reshape(GT, P, GT, P)

    in_maps = []
    for c in range(NCORES):
        rows0 = c * RPC
        # at[rt, p_dst, kt, f_src]; partition-major contiguous per rt
        at = np.ascontiguousarray(
            A4[c * RT:(c + 1) * RT].transpose(0, 3, 2, 1))       # [RT,P,GT,P]
        xa = np.zeros((RT, P, FA), dtype=np.float32)
        xa[:, :, :F] = x_zero[rows0:rows0 + RPC].reshape(RT, P, F)
        xa[:, :, F] = atte_zero[rows0:rows0 + RPC].reshape(RT, P)
        ell_own = np.ascontiguousarray(
            ell[rows0:rows0 + RPC].reshape(RT, P, deg_pad)
            .transpose(1, 0, 2)).astype(np.float16)
        in_maps.append({
            "at": at,
            "xat": xat.astype(np.float16),
            "ell": ell_dev,
            "ellown": ell_own,
            "xa": xa,
        })
    return in_maps, deg_pad


# ---------------------------------------------------------------------------
# Device program
# ---------------------------------------------------------------------------

def build_program(deg_pad, trn_type="TRN2"):
    nc = bacc.Bacc(trn_type, target_bir_lowering=False, debug=False)

    at = nc.dram_tensor("at", [RT, P, GT, P], FP16, kind="ExternalInput")
    xat = nc.dram_tensor("xat", [P, GT, FA], FP16, kind="ExternalInput")
    ell = nc.dram_tensor("ell", [P, GT, deg_pad], FP16, kind="ExternalInput")
    ellown = nc.dram_tensor("ellown", [P, RT, deg_pad], FP16, kind="ExternalInput")
    xa = nc.dram_tensor("xa", [RT, P, FA], FP32, kind="ExternalInput")
    out = nc.dram_tensor("out", [RT, P, FA], FP32, kind="ExternalOutput")

    with tile.TileContext(nc) as tc:
        _build(tc, nc, deg_pad, at, xat, ell, ellown, xa, out)
    nc.compile()
    return nc


def _build(tc, nc, deg_pad, at, xat, ell, ellown, xa, out):
    import contextlib
    XCH = 8                 # xat load chunks
    TC = GT // XCH          # t-columns per chunk
    ctx = contextlib.ExitStack()
    with ctx:
        cpool = ctx.enter_context(tc.tile_pool(name="consts", bufs=1))
        xpool = ctx.enter_context(tc.tile_pool(name="xin", bufs=2))
        apool = ctx.enter_context(tc.tile_pool(name="atiles", bufs=5))
        epool = ctx.enter_context(tc.tile_pool(name="epi", bufs=3))
        pspool = ctx.enter_context(tc.tile_pool(name="ps", bufs=2, space="PSUM"))

        # ---- A^T streaming on the ACT HWDGE ring, issued first ----
        at_tiles = []
        for rt in range(RT):
            att = apool.tile([P, GT, P], FP16, name=f"at{rt}", tag="at")
            nc.scalar.dma_start(att[:], at[rt, :, :, :])
            at_tiles.append(att)

        # ---- dinv + SBUF-resident z = [x|atte]*dinv, pipelined per chunk ----
        # Everything chunked by TC node-tiles so the first matmuls can start
        # as soon as the first z tiles exist (z is one tile per node-tile).
        z_tiles = []
        dinv_chunks = []
        for ch in range(XCH):
            t0 = ch * TC
            ec = xpool.tile([P, TC, deg_pad], FP16, tag="ec")
            nc.sync.dma_start(ec[:], ell[:, t0:t0 + TC, :])
            xc = xpool.tile([P, TC, FA], FP16, tag="xc")
            nc.sync.dma_start(xc[:], xat[:, t0:t0 + TC, :])
            rsc = cpool.tile([P, TC], FP32, name=f"rs{ch}", tag="rs", bufs=1)
            nc.vector.tensor_reduce(rsc[:], ec[:], axis=mybir.AxisListType.X,
                                    op=mybir.AluOpType.add)
            sqc = cpool.tile([P, TC], FP32, name=f"sq{ch}", tag="sqc", bufs=1)
            nc.scalar.activation(sqc[:], rsc[:],
                                 mybir.ActivationFunctionType.Sqrt,
                                 bias=1.0, scale=1.0)
            dc = cpool.tile([P, TC], FP32, name=f"dc{ch}", tag="dc", bufs=1)
            nc.vector.reciprocal(dc[:], sqc[:])
            dinv_chunks.append(dc)
            for j in range(TC):
                zt = cpool.tile([P, FA], FP16, name=f"z{t0 + j}", tag=f"z{t0 + j}")
                nc.vector.tensor_scalar(
                    out=zt[:], in0=xc[:, j, :],
                    scalar1=dc[:, j:j + 1], scalar2=None,
                    op0=mybir.AluOpType.mult)
                z_tiles.append(zt)

        # ---- own-row dinv (row-tile-major) for the epilogue ----
        ellown_sb = cpool.tile([P, RT, deg_pad], FP16)
        nc.sync.dma_start(ellown_sb[:], ellown[:])
        rs_own = cpool.tile([P, RT], FP32)
        nc.vector.tensor_reduce(rs_own[:], ellown_sb[:], axis=mybir.AxisListType.X,
                                op=mybir.AluOpType.add)
        sq_own = cpool.tile([P, RT], FP32)
        nc.scalar.activation(sq_own[:], rs_own[:],
                             mybir.ActivationFunctionType.Sqrt, bias=1.0, scale=1.0)
        own_d = cpool.tile([P, RT], FP32)
        nc.vector.reciprocal(own_d[:], sq_own[:])
        own_dd = cpool.tile([P, RT], FP32)
        nc.vector.tensor_tensor(out=own_dd[:], in0=own_d[:], in1=own_d[:],
                                op=mybir.AluOpType.mult)

        # ---- per row tile: 96-deep PSUM accumulation + epilogue ----
        for rt in range(RT):
            att = at_tiles[rt]
            ps = pspool.tile([P, FA], FP32, tag="ps")
            for kt in range(GT):
                nc.tensor.matmul(ps[:], lhsT=att[:, kt, :], rhs=z_tiles[kt][:],
                                 start=(kt == 0), stop=(kt == GT - 1))

            # out = dinv_r * psum + dinv_r^2 * xa   (all fp32)
            xa_t = epool.tile([P, FA], FP32, tag="xa")
            nc.sync.dma_start(xa_t[:], xa[rt, :, :])
            o1 = epool.tile([P, FA], FP32, tag="o1")
            nc.vector.tensor_scalar(out=o1[:], in0=ps[:],
                                    scalar1=own_d[:, rt:rt + 1],
                                    scalar2=None, op0=mybir.AluOpType.mult)
            o2 = epool.tile([P, FA], FP32, tag="o2")
            nc.vector.tensor_scalar(out=o2[:], in0=xa_t[:],
                                    scalar1=own_dd[:, rt:rt + 1],
                                    scalar2=None, op0=mybir.AluOpType.mult)
            res = epool.tile([P, FA], FP32, tag="res")
            nc.vector.tensor_tensor(out=res[:], in0=o1[:], in1=o2[:],
                                    op=mybir.AluOpType.add)
            nc.sync.dma_start(out[rt, :, :], res[:])


# ---------------------------------------------------------------------------
# Entry point
# ---------------------------------------------------------------------------

def kernel(fea, perm, encoder_edge_index, encoder_edge_attr, node_atte_coffe,
           all_node_num, batch_size):
    global LAST_RESULTS
    in_maps, deg_pad = host_prep(
        fea, perm, encoder_edge_index, encoder_edge_attr, node_atte_coffe,
        all_node_num, batch_size)
    nc = build_program(deg_pad)
    res = run_bass_kernel_spmd(nc, in_maps, core_ids=list(range(NCORES)),
                               **RUN_KWARGS)
    LAST_RESULTS = res
    x = np.zeros((N, F), dtype=np.float32)
    atte = np.zeros((N,), dtype=np.float32)
    for c in range(NCORES):
        o = res.results[c]["out"].reshape(RPC, FA)
        x[c * RPC:(c + 1) * RPC] = o[:, :F]
        atte[c * RPC:(c + 1) * RPC] = o[:, F]
    return x, atte


# revision 19
# speedup vs baseline: 1.0717x; 1.0717x over previous
"""Trainium2 Bass kernel for DiffusioUnpool (gnn_message_passing).

Computes, for a graph with N=12288 nodes, F=128 features, E=393216 COO edges:
    x_zero    = scatter(fea via perm)                     [N, F]
    atte_zero = scatter(tiled node_atte_coffe via perm)   [N]
    A         = coo_sum(edges) + I                        (dense adjacency)
    dinv      = 1/sqrt(A.sum(axis=1))
    x         = dinv * (A @ (x_zero * dinv))
    atte      = dinv * (A @ (atte_zero * dinv))

Strategy (row-sharded across 8 cores, 1536 rows each):
    out[r] = dinv_r * sum_dst A[r,dst] * z[dst]  +  dinv_r^2 * xa[r]
with z = [x_zero | atte_zero] * dinv built on-device in fp16 and kept
SBUF-resident (3.2MB), and the dense A^T streamed from DRAM in fp16 as the
matmul stationary operand: per output row-tile, one contiguous 3.1MB DMA
brings all 96 [128x128] A^T K-tiles, and the PE accumulates 96 fp16 matmuls
(N=129) into a fp32 PSUM bank.  Row sums (degree+1) are reduced on-device
from a padded ELL layout of attr; the epilogue rescales by the row dinv and
adds the identity term in fp32.  This keeps the kernel entirely on the
HWDGE DMA path + PE: no software-DGE gathers (the Q7 descriptor-generation
rate, ~8ns/descriptor, made per-edge gathering the bottleneck).
"""

import os
import sys

import numpy as np

for _p in ("/opt/trn_rl_repo", "/root/.axon_site/_ro/trn_rl_repo"):
    if os.path.isdir(_p) and _p not in sys.path:
        sys.path.append(_p)

import concourse.bacc as bacc
import concourse.bass as bass
import concourse.mybir as mybir
import concourse.tile as tile
from concourse.bass_utils import run_bass_kernel_spmd

FP32 = mybir.dt.float32
FP16 = mybir.dt.float16

N = 12288          # all_node_num
F = 128            # feature dim
FA = F + 1         # features + atte channel
NCORES = 8
P = 128            # partitions
RPC = N // NCORES  # rows per core = 1536
RT = RPC // P      # row tiles per core = 12
GT = N // P        # node tiles = 96

# Stash of the last BassKernelResults (test.py reads .exec_time_ns)
LAST_RESULTS = None
# Extra kwargs test.py can inject into run_bass_kernel_spmd (e.g. trace)
RUN_KWARGS = {}


# ---------------------------------------------------------------------------
# Host-side preparation: scatter, dense A^T tiles, ELL degree layout
# ---------------------------------------------------------------------------

def host_prep(fea, perm, encoder_edge_index, encoder_edge_attr, node_atte_coffe,
              all_node_num, batch_size):
    n = int(all_node_num)
    b = int(batch_size)
    assert n == N
    fea = np.asarray(fea, dtype=np.float32)
    perm = np.asarray(perm).astype(np.int64)
    eidx = np.asarray(encoder_edge_index).astype(np.int64)
    attr = np.asarray(encoder_edge_attr, dtype=np.float32)
    natte = np.asarray(node_atte_coffe, dtype=np.float32)

    n_perm, f = fea.shape
    assert f == F
    node_num = natte.shape[0] // b
    swn = n_perm // natte.shape[0]

    # unpool scatters
    x_zero = np.zeros((N, F), dtype=np.float32)
    x_zero[perm] = fea
    win = np.broadcast_to(natte.reshape(b, 1, node_num),
                          (b, swn, node_num)).reshape(-1).astype(np.float32)
    atte_zero = np.zeros((N,), dtype=np.float32)
    atte_zero[perm] = win

    src = eidx[0]
    dst = eidx[1]
    E = src.shape[0]

    # --- ELL attr layout for on-device row sums (t-major: g = t*P + p) ---
    deg = np.bincount(src, minlength=N)
    deg_pad = max(4, int(-(-int(deg.max()) // 8) * 8))
    o1 = np.argsort(src, kind="stable")
    ssrc = src[o1]
    row_starts = np.zeros(N, dtype=np.int64)
    row_starts[1:] = np.cumsum(deg)[:-1]
    pos1 = np.arange(E) - row_starts[ssrc]
    ell = np.zeros((N, deg_pad), dtype=np.float32)
    ell[ssrc, pos1] = attr[o1]
    ell_dev = np.ascontiguousarray(
        ell.reshape(GT, P, deg_pad).transpose(1, 0, 2)).astype(np.float16)

    # [x_zero | atte_zero] in t-major node-tile layout for the z build
    xat = np.zeros((P, GT, FA), dtype=np.float32)
    xat[:, :, :F] = x_zero.reshape(GT, P, F).transpose(1, 0, 2)
    xat[:, :, F] = atte_zero.reshape(GT, P).T

    # --- dense A^T, fp16, tiled per core: at[rt, p_dst, kt, f_src] ---
    A32 = np.zeros((N, N), dtype=np.float32)
    np.add.at(A32, (src, dst), attr)
    A16 = A32.astype(np.float16)
    del A32
    # [rt_g, f_src, kt, p_dst] view of A[row, dst]
    A4 = A16.reshape(GT, P, GT, P)

    in_maps = []
    for c in range(NCORES):
        rows0 = c * RPC
        # at[rt, p_dst, kt, f_src]; partition-major contiguous per rt
        at = np.ascontiguousarray(
            A4[c * RT:(c + 1) * RT].transpose(0, 3, 2, 1))       # [RT,P,GT,P]
        xa = np.zeros((RT, P, FA), dtype=np.float32)
        xa[:, :, :F] = x_zero[rows0:rows0 + RPC].reshape(RT, P, F)
        xa[:, :, F] = atte_zero[rows0:rows0 + RPC].reshape(RT, P)
        ell_own = np.ascontiguousarray(
            ell[rows0:rows0 + RPC].reshape(RT, P, deg_pad)
            .transpose(1, 0, 2)).astype(np.float16)
        in_maps.append({
            "at": at,
            "xat": xat.astype(np.float16),
            "ell": ell_dev,
            "ellown": ell_own,
            "xa": xa,
        })
    return in_maps, deg_pad


# ---------------------------------------------------------------------------
# Device program
# ---------------------------------------------------------------------------

def build_program(deg_pad, trn_type="TRN2"):
    nc = bacc.Bacc(trn_type, target_bir_lowering=False, debug=False)

    at = nc.dram_tensor("at", [RT, P, GT, P], FP16, kind="ExternalInput")
    xat = nc.dram_tensor("xat", [P, GT, FA], FP16, kind="ExternalInput")
    ell = nc.dram_tensor("ell", [P, GT, deg_pad], FP16, kind="ExternalInput")
    ellown = nc.dram_tensor("ellown", [P, RT, deg_pad], FP16, kind="ExternalInput")
    xa = nc.dram_tensor("xa", [RT, P, FA], FP32, kind="ExternalInput")
    out = nc.dram_tensor("out", [RT, P, FA], FP32, kind="ExternalOutput")

    with tile.TileContext(nc) as tc:
        _build(tc, nc, deg_pad, at, xat, ell, ellown, xa, out)
    nc.compile()
    return nc


def _build(tc, nc, deg_pad, at, xat, ell, ellown, xa, out):
    import contextlib
    XCH = 8                 # xat load chunks
    TC = GT // XCH          # t-columns per chunk
    ctx = contextlib.ExitStack()
    with ctx:
        cpool = ctx.enter_context(tc.tile_pool(name="consts", bufs=1))
        xpool = ctx.enter_context(tc.tile_pool(name="xin", bufs=2))
        apool = ctx.enter_context(tc.tile_pool(name="atiles", bufs=10))
        epool = ctx.enter_context(tc.tile_pool(name="epi", bufs=3))
        pspool = ctx.enter_context(tc.tile_pool(name="ps", bufs=2, space="PSUM"))

        # ---- A^T streaming, half-tiles on both HWDGE rings, issued first ----
        HG = GT // 2
        at_halves = []
        for h in range(2 * RT):
            rt, lo = divmod(h, 2)
            ath = apool.tile([P, HG, P], FP16, name=f"at{h}", tag="at")
            eng = nc.scalar if h % 2 == 0 else nc.sync
            eng.dma_start(ath[:], at[rt, :, lo * HG:(lo + 1) * HG, :])
            at_halves.append(ath)

        # ---- dinv + SBUF-resident z = [x|atte]*dinv, pipelined per chunk ----
        # Everything chunked by TC node-tiles so the first matmuls can start
        # as soon as the first z tiles exist (z is one tile per node-tile).
        z_tiles = []
        dinv_chunks = []
        for ch in range(XCH):
            t0 = ch * TC
            ec = xpool.tile([P, TC, deg_pad], FP16, tag="ec")
            nc.sync.dma_start(ec[:], ell[:, t0:t0 + TC, :])
            xc = xpool.tile([P, TC, FA], FP16, tag="xc")
            nc.sync.dma_start(xc[:], xat[:, t0:t0 + TC, :])
            rsc = cpool.tile([P, TC], FP32, name=f"rs{ch}", tag="rs", bufs=1)
            nc.vector.tensor_reduce(rsc[:], ec[:], axis=mybir.AxisListType.X,
                                    op=mybir.AluOpType.add)
            sqc = cpool.tile([P, TC], FP32, name=f"sq{ch}", tag="sqc", bufs=1)
            nc.scalar.activation(sqc[:], rsc[:],
                                 mybir.ActivationFunctionType.Sqrt,
                                 bias=1.0, scale=1.0)
            dc = cpool.tile([P, TC], FP32, name=f"dc{ch}", tag="dc", bufs=1)
            nc.vector.reciprocal(dc[:], sqc[:])
            dinv_chunks.append(dc)
            for j in range(TC):
                zt = cpool.tile([P, FA], FP16, name=f"z{t0 + j}", tag=f"z{t0 + j}")
                nc.vector.tensor_scalar(
                    out=zt[:], in0=xc[:, j, :],
                    scalar1=dc[:, j:j + 1], scalar2=None,
                    op0=mybir.AluOpType.mult)
                z_tiles.append(zt)

        # ---- own-row dinv (row-tile-major) for the epilogue ----
        ellown_sb = cpool.tile([P, RT, deg_pad], FP16)
        nc.sync.dma_start(ellown_sb[:], ellown[:])
        rs_own = cpool.tile([P, RT], FP32)
        nc.vector.tensor_reduce(rs_own[:], ellown_sb[:], axis=mybir.AxisListType.X,
                                op=mybir.AluOpType.add)
        sq_own = cpool.tile([P, RT], FP32)
        nc.scalar.activation(sq_own[:], rs_own[:],
                             mybir.ActivationFunctionType.Sqrt, bias=1.0, scale=1.0)
        own_d = cpool.tile([P, RT], FP32)
        nc.vector.reciprocal(own_d[:], sq_own[:])
        own_dd = cpool.tile([P, RT], FP32)
        nc.vector.tensor_tensor(out=own_dd[:], in0=own_d[:], in1=own_d[:],
                                op=mybir.AluOpType.mult)

        # ---- per row tile: 96-deep PSUM accumulation + epilogue ----
        for rt in range(RT):
            ps = pspool.tile([P, FA], FP32, tag="ps")
            for kt in range(GT):
                ath = at_halves[2 * rt + kt // HG]
                nc.tensor.matmul(ps[:], lhsT=ath[:, kt % HG, :],
                                 rhs=z_tiles[kt][:],
                                 start=(kt == 0), stop=(kt == GT - 1))

            # out = dinv_r * psum + dinv_r^2 * xa   (all fp32)
            xa_t = epool.tile([P, FA], FP32, tag="xa")
            nc.sync.dma_start(xa_t[:], xa[rt, :, :])
            o1 = epool.tile([P, FA], FP32, tag="o1")
            nc.vector.tensor_scalar(out=o1[:], in0=ps[:],
                                    scalar1=own_d[:, rt:rt + 1],
                                    scalar2=None, op0=mybir.AluOpType.mult)
            o2 = epool.tile([P, FA], FP32, tag="o2")
            nc.vector.tensor_scalar(out=o2[:], in0=xa_t[:],
                                    scalar1=own_dd[:, rt:rt + 1],
                                    scalar2=None, op0=mybir.AluOpType.mult)
            res = epool.tile([P, FA], FP32, tag="res")
            nc.vector.tensor_tensor(out=res[:], in0=o1[:], in1=o2[:],
                                    op=mybir.AluOpType.add)
            nc.sync.dma_start(out[rt, :, :], res[:])


# ---------------------------------------------------------------------------
# Entry point
# ---------------------------------------------------------------------------

def kernel(fea, perm, encoder_edge_index, encoder_edge_attr, node_atte_coffe,
           all_node_num, batch_size):
    global LAST_RESULTS
    in_maps, deg_pad = host_prep(
        fea, perm, encoder_edge_index, encoder_edge_attr, node_atte_coffe,
        all_node_num, batch_size)
    nc = build_program(deg_pad)
    res = run_bass_kernel_spmd(nc, in_maps, core_ids=list(range(NCORES)),
                               **RUN_KWARGS)
    LAST_RESULTS = res
    x = np.zeros((N, F), dtype=np.float32)
    atte = np.zeros((N,), dtype=np.float32)
    for c in range(NCORES):
        o = res.results[c]["out"].reshape(RPC, FA)
        x[c * RPC:(c + 1) * RPC] = o[:, :F]
        atte[c * RPC:(c + 1) * RPC] = o[:, F]
    return x, atte
